# revision 12
# baseline (speedup 1.0000x reference)
"""GalaxyTileDecoder on 8 Trainium2 NeuronCores — sorted slab-pair-tile scheme.

Same algebraic collapse as before (decode+pad+gate+bilinear-shift+sum ==
one matmul z_exp @ W_exp with K = 6*6*9 = 324), but instead of streaming
all three 128-row K-chunks for every batch, ptiles are SORTED by the
integer y-shifts (m = floor(2.5 - 4*locs_y) per source, w = m+2 in 0..4).
A source only touches W rows in two adjacent 54-row "a-slabs" {w, w+1},
i.e. one of five 108-row pair tiles T_w = W_exp[54w : 54w+108].  Batches
("slots") of <=128 ptiles whose sources fit a small set of pair tiles
stream only those tiles: ~2.1 passes of 2704 columns per slot instead of
3, cutting PE streaming ~30%.

All 8 cores run one SPMD program, so the slot structure (how many slots,
which tiles each streams) is computed on the host for the actual data
(max-flow feasibility across all cores), and the bass program is built &
cached per structure.  Each slot is padded to 128 rows; the host
un-permutes the row order after gather.
"""

import os
from collections import defaultdict

import numpy as np

P_TOTAL = 10000
M = 2
N_CORES = 8
PT = P_TOTAL // N_CORES
F = 9
A = 6
B = 6
K = A * B * F                    # 324
TK = 108                         # rows per pair tile
OUT_HW = 52
COLS = OUT_HW * OUT_HW           # 2704
CANVAS = 57
NTILES = 5                       # pair tiles T_0..T_4

WARMUP_MM = 60

PIECES = [(0, 1024), (1024, 2048), (2048, COLS)]
SEGS = {0: [(0, 512), (512, 1024)],
        1: [(1024, 1536), (1536, 2048)],
        2: [(2048, 2560), (2560, COLS)]}

_DT_NAME = os.environ.get("BASS_GAL_DT", "bf16")

_cache = {}


# ----------------------------------------------------------------- host math

def _source_coeffs(locs, galaxy_params, galaxy_bool):
    locs = np.asarray(locs, np.float32).reshape(-1, 2)
    params = np.asarray(galaxy_params, np.float32).reshape(-1, 8)
    gbool = np.asarray(galaxy_bool, np.float32).reshape(-1, 1)
    N = locs.shape[0]
    sy = 2.5 - 4.0 * locs[:, 0]
    sx = 2.5 - 4.0 * locs[:, 1]
    m = np.floor(sy)
    kx = np.floor(sx)
    fy = (sy - m).astype(np.float32)
    fx = (sx - kx).astype(np.float32)
    w = (m.astype(np.int64) + 2)          # 0..4 tile id per source
    bx = (kx.astype(np.int64) + 2)        # 0..4 (b taps bx, bx+1)
    z9 = np.concatenate([params, np.ones((N, 1), np.float32)], 1) * gbool
    wy = np.stack([1.0 - fy, fy], 1)
    wx = np.stack([1.0 - fx, fx], 1)
    coeff = wy[:, :, None, None] * wx[:, None, :, None] * z9[:, None, None, :]
    return (w.reshape(P_TOTAL, M), bx.reshape(P_TOTAL, M),
            coeff.reshape(P_TOTAL, M, 2, 2, F).astype(np.float32))


def _build_wexp(W_dec, b_dec, np_dtype):
    canvas9 = np.zeros((F, CANVAS, CANVAS), np.float32)
    canvas9[:8, 3:54, 3:54] = np.asarray(W_dec, np.float32).reshape(8, 51, 51)
    canvas9[8, 3:54, 3:54] = np.asarray(b_dec, np.float32).reshape(51, 51)
    sw = np.lib.stride_tricks.sliding_window_view(
        canvas9, (OUT_HW, OUT_HW), axis=(1, 2))
    return np.ascontiguousarray(
        sw.transpose(1, 2, 0, 3, 4).reshape(K, COLS), dtype=np_dtype)


# --------------------------------------------------------------- slot packing

def _pack_core(wpair):
    """Greedy per-core packing into slots of <=128 rows, slots labeled by
    tile-sets; merges partial slots while the union stays <=3 tiles."""
    groups = defaultdict(list)
    for i in range(wpair.shape[0]):
        groups[frozenset(wpair[i].tolist())].append(i)
    slots = []
    frags = []
    for key, rows in sorted(groups.items(), key=lambda kv: -len(kv[1])):
        k = 0
        while len(rows) - k >= 128:
            slots.append((set(key), rows[k:k + 128])); k += 128
        if rows[k:]:
            frags.append((set(key), rows[k:]))

    def merge_pass():
        best = None
        for x in range(len(frags)):
            for y in range(x + 1, len(frags)):
                t1, r1 = frags[x]; t2, r2 = frags[y]
                if len(r1) + len(r2) > 128:
                    continue
                u = t1 | t2
                if len(u) > 3:
                    continue
                gain = len(t1) + len(t2) - len(u)
                if gain <= 0:
                    continue
                fill = len(r1) + len(r2)
                if best is None or (gain, fill) > best[0]:
                    best = ((gain, fill), x, y)
        if best is None:
            return False
        _, x, y = best
        t1, r1 = frags[x]; t2, r2 = frags[y]
        nf = (t1 | t2, r1 + r2)
        for idx in sorted((x, y), reverse=True):
            frags.pop(idx)
        frags.append(nf)
        return True

    while merge_pass():
        pass
    slots.extend(frags)
    return slots


def _flow_feasible(demands, slots):
    """demands: dict row-typeset -> count; slots: list of tile-sets.
    Returns assignment dict (typeset -> {slot_idx: n}) or None."""
    from scipy.sparse import csr_matrix
    from scipy.sparse.csgraph import maximum_flow
    types = sorted(demands, key=sorted)
    T, S = len(types), len(slots)
    n = 2 + T + S
    src, snk = 0, n - 1
    rows, cols, caps = [], [], []
    total = 0
    for ti, ty in enumerate(types):
        rows.append(src); cols.append(1 + ti); caps.append(demands[ty])
        total += demands[ty]
        for si, ss in enumerate(slots):
            if ty <= ss:
                rows.append(1 + ti); cols.append(1 + T + si); caps.append(128)
    for si in range(S):
        rows.append(1 + T + si); cols.append(snk); caps.append(128)
    g = csr_matrix((caps, (rows, cols)), shape=(n, n), dtype=np.int32)
    r = maximum_flow(g, src, snk)
    if r.flow_value != total:
        return None
    fl = r.flow.tocoo()
    assign = defaultdict(dict)
    for i, j, v in zip(fl.row, fl.col, fl.data):
        if 1 <= i <= T and 1 + T <= j < 1 + T + S and v > 0:
            assign[types[i - 1]][j - 1 - T] = int(v)
    return assign


def _common_structure(w):
    """Compute a common slot structure feasible for all cores, and per-core
    row assignments.  Returns (slots, per_core_rows) where slots is a list of
    sorted tile tuples and per_core_rows[c][k] is the row list of slot k."""
    per_core_slots = []
    demands = []
    for c in range(N_CORES):
        wp = w[c * PT:(c + 1) * PT]
        per_core_slots.append(_pack_core(wp))
        d = defaultdict(int)
        for i in range(PT):
            d[frozenset(wp[i].tolist())] += 1
        demands.append(dict(d))

    # initial common multiset: per-set max count across cores
    cnt = defaultdict(int)
    for sl in per_core_slots:
        c = defaultdict(int)
        for ts, _ in sl:
            c[frozenset(ts)] += 1
        for k, v in c.items():
            cnt[k] = max(cnt[k], v)
    slots = []
    for ts, v in cnt.items():
        slots.extend([set(ts)] * v)

    def all_feasible(cand):
        return all(_flow_feasible(demands[c], cand) is not None
                   for c in range(N_CORES))

    assert all_feasible(slots), "initial common structure infeasible"

    # greedy shrink: try removing slots (most expensive first), then try
    # merging slot pairs into their union when that reduces total passes
    improved = True
    while improved:
        improved = False
        order = sorted(range(len(slots)), key=lambda i: -len(slots[i]))
        for i in order:
            cand = slots[:i] + slots[i + 1:]
            if all_feasible(cand):
                slots = cand
                improved = True
                break
        if improved:
            continue
        nS = len(slots)
        for x in range(nS):
            for y in range(x + 1, nS):
                u = slots[x] | slots[y]
                if len(u) >= len(slots[x]) + len(slots[y]) or len(u) > 3:
                    continue
                cand = [slots[i] for i in range(nS) if i not in (x, y)] + [u]
                if all_feasible(cand):
                    slots = cand
                    improved = True
                    break
            if improved:
                break

    slots = sorted(slots, key=lambda s: (len(s), sorted(s)))
    # per-core concrete row assignment
    per_core_rows = []
    for c in range(N_CORES):
        assign = _flow_feasible(demands[c], slots)
        assert assign is not None
        wp = w[c * PT:(c + 1) * PT]
        by_type = defaultdict(list)
        for i in range(PT):
            by_type[frozenset(wp[i].tolist())].append(i)
        rows_per_slot = [[] for _ in slots]
        for ty, fl in assign.items():
            pool = by_type[ty]
            pos = 0
            for si, v in sorted(fl.items()):
                rows_per_slot[si].extend(pool[pos:pos + v])
                pos += v
            assert pos == len(pool)
        per_core_rows.append(rows_per_slot)
    return [tuple(sorted(s)) for s in slots], per_core_rows


# ------------------------------------------------------------- bass program

def _build_program(dt_name, slots):
    import concourse.bass as bass  # noqa: F401
    import concourse.tile as tile
    from concourse import bacc, mybir

    DT = {"bf16": mybir.dt.bfloat16, "f32": mybir.dt.float32}[dt_name]
    n_slots = len(slots)
    n_blocks = sum(len(s) for s in slots)

    nc = bacc.Bacc(trn_type="TRN2")
    zt = nc.dram_tensor("zt", [TK, n_blocks * 128], DT, kind="ExternalInput")
    w_dram = {}
    for t in range(NTILES):
        for pi, (p0, p1) in enumerate(PIECES):
            w_dram[t, pi] = nc.dram_tensor(
                f"w{t}_{pi}", [TK, p1 - p0], DT, kind="ExternalInput")
    out = nc.dram_tensor("out", [n_slots * 128, COLS], DT,
                         kind="ExternalOutput")

    # block index per (slot, tile-within-slot)
    blk = {}
    bcur = 0
    for si, s in enumerate(slots):
        for j, t in enumerate(s):
            blk[si, j] = bcur
            bcur += 1

    # first-use order of tiles (for load ordering)
    tile_order = []
    for s in slots:
        for t in s:
            if t not in tile_order:
                tile_order.append(t)
    for t in range(NTILES):
        if t not in tile_order:
            tile_order.append(t)

    with tile.TileContext(nc) as tc:
        with (
            tc.tile_pool(name="w", bufs=1) as wpool,
            tc.tile_pool(name="o", bufs=6) as opool,
            tc.tile_pool(name="ps", bufs=3, space="PSUM") as pspool,
            tc.tile_pool(name="wm", bufs=1, space="PSUM") as wmpool,
        ):
            warm = wpool.tile([128, 128], mybir.dt.bfloat16, tag="warm")
            nc.vector.memset(warm[:], 0.0)
            wps = wmpool.tile([128, 64], mybir.dt.float32, tag="warmps")
            for _ in range(WARMUP_MM):
                nc.tensor.matmul(wps[:, :], warm[:, 0:128], warm[:, 0:64],
                                 start=True, stop=True)

            # All loads on the sync HWDGE queue, interleaved in first-use
            # order: early z blocks and the piece-0 W tiles the first slots
            # stream, then the rest.
            ZCH = 6                                 # z blocks per load chunk
            z_chunks = []                           # (b_lo, tile)
            w_tiles = {}

            def load_z_chunk(b_lo):
                b_hi = min(b_lo + ZCH, n_blocks)
                ztile = wpool.tile([TK, (b_hi - b_lo) * 128], DT,
                                   tag=f"z{b_lo}")
                nc.sync.dma_start(ztile[:], zt[:, b_lo * 128:b_hi * 128])
                z_chunks.append((b_lo, b_hi, ztile))

            load_z_chunk(0)
            nw = 0
            for t in tile_order[:2]:
                wt = wpool.tile([TK, 1024], DT, tag=f"w{t}_0")
                nc.sync.dma_start(wt[:], w_dram[t, 0][:, :])
                w_tiles[t, 0] = wt
                nw += 1
            if n_blocks > ZCH:
                load_z_chunk(ZCH)
            for t in tile_order[2:]:
                wt = wpool.tile([TK, 1024], DT, tag=f"w{t}_0")
                nc.sync.dma_start(wt[:], w_dram[t, 0][:, :])
                w_tiles[t, 0] = wt
            for b_lo in range(2 * ZCH, n_blocks, ZCH):
                load_z_chunk(b_lo)
            # piece-1/2 weights ride the scalar HWDGE queue, which is idle
            # during the load phase, so they land well before piece 0 ends.
            for pi in range(1, len(PIECES)):
                p0, p1 = PIECES[pi]
                for t in tile_order:
                    wt = wpool.tile([TK, p1 - p0], DT, tag=f"w{t}_{pi}")
                    nc.scalar.dma_start(wt[:], w_dram[t, pi][:, :])
                    w_tiles[t, pi] = wt

            def zcols(si, j):
                b = blk[si, j]
                for b_lo, b_hi, ztile in z_chunks:
                    if b_lo <= b < b_hi:
                        return ztile[:, (b - b_lo) * 128:(b - b_lo + 1) * 128]
                raise AssertionError(b)

            eng = 0
            for pi, (p0, p1) in enumerate(PIECES):
                pw = p1 - p0
                for si, s in enumerate(slots):
                    b0 = si * 128
                    ps = pspool.tile([128, 1024], mybir.dt.float32, tag="ps")
                    for j, t in enumerate(s):
                        for (s0, s1) in SEGS[pi]:
                            nc.tensor.matmul(
                                ps[:, s0 - p0:s1 - p0],
                                zcols(si, j),
                                w_tiles[t, pi][:, s0 - p0:s1 - p0],
                                start=(j == 0),
                                stop=(j == len(s) - 1),
                            )
                    lastsl = (pi == len(PIECES) - 1) and (si == n_slots - 1)
                    spans = ([(s0 - p0, s1 - p0) for (s0, s1) in SEGS[pi]]
                             if lastsl else [(0, pw)])
                    for (c0, c1) in spans:
                        osb = opool.tile([128, 1024], DT, tag="osb")
                        if eng == 0:
                            nc.vector.tensor_copy(osb[:, 0:c1 - c0],
                                                  ps[:, c0:c1])
                            nc.sync.dma_start(out[b0:b0 + 128,
                                                  p0 + c0:p0 + c1],
                                              osb[:, 0:c1 - c0])
                        else:
                            nc.scalar.copy(osb[:, 0:c1 - c0], ps[:, c0:c1])
                            nc.scalar.dma_start(out[b0:b0 + 128,
                                                    p0 + c0:p0 + c1],
                                                osb[:, 0:c1 - c0])
                        eng ^= 1
    nc.compile()
    return nc


def _get_program(dt_name, slots):
    key = (dt_name, tuple(slots))
    if key not in _cache:
        _cache[key] = _build_program(dt_name, slots)
    return _cache[key]


# ------------------------------------------------------------------- kernel

def kernel(locs, galaxy_params, galaxy_bool, W_dec, b_dec, _trace=False):
    import ml_dtypes
    from concourse.bass_utils import run_bass_kernel_spmd

    np_dtype = {"bf16": ml_dtypes.bfloat16, "f32": np.float32}[_DT_NAME]

    w, bx, coeff = _source_coeffs(locs, galaxy_params, galaxy_bool)
    Wexp = _build_wexp(W_dec, b_dec, np_dtype)
    slots, per_core_rows = _common_structure(w)
    n_slots = len(slots)
    n_blocks = sum(len(s) for s in slots)

    # z blocks: (core, TK, n_blocks*128)
    zt = np.zeros((N_CORES, TK, n_blocks * 128), np.float32)
    row_maps = []          # per core: array of ptile ids per (slot, slot_row)
    for c in range(N_CORES):
        rows_per_slot = per_core_rows[c]
        rmap = np.full(n_slots * 128, -1, np.int64)
        bcur = 0
        for si, s in enumerate(slots):
            rows = rows_per_slot[si]
            for r, p in enumerate(rows):
                rmap[si * 128 + r] = c * PT + p
            for j, t in enumerate(s):
                col0 = bcur * 128
                for r, p in enumerate(rows):
                    gp = c * PT + p
                    for src in range(M):
                        if w[gp, src] != t:
                            continue
                        cf = coeff[gp, src]          # (2, 2, F)
                        bxx = bx[gp, src]
                        for aoff in range(2):
                            base = aoff * 54 + bxx * 9
                            zt[c, base:base + F, col0 + r] += cf[aoff, 0]
                            zt[c, base + 9:base + 9 + F, col0 + r] += cf[aoff, 1]
                bcur += 1
        row_maps.append(rmap)

    nc = _get_program(_DT_NAME, slots)
    in_maps = []
    for c in range(N_CORES):
        m_ = {"zt": zt[c].astype(np_dtype)}
        for t in range(NTILES):
            for pi, (p0, p1) in enumerate(PIECES):
                m_[f"w{t}_{pi}"] = np.ascontiguousarray(
                    Wexp[54 * t:54 * t + TK, p0:p1])
        in_maps.append(m_)
    kwargs = {"trace": True} if _trace else {}
    res = run_bass_kernel_spmd(nc, in_maps, core_ids=list(range(N_CORES)),
                               **kwargs)

    out = np.zeros((P_TOTAL, COLS), np.float32)
    for c in range(N_CORES):
        o = np.asarray(res.results[c]["out"]).astype(np.float32)
        rmap = row_maps[c]
        valid = rmap >= 0
        out[rmap[valid]] = o[valid]
    out = out.reshape(P_TOTAL, 1, OUT_HW, OUT_HW)
    if _trace:
        kernel._last_result = res
    return out, out
